# revision 5
# baseline (speedup 1.0000x reference)
"""Bidirectional Vim-Mamba2 encoder on 8 Trainium2 NeuronCores.

Sharding: core c -> (sample b = c//2, direction d = c%2). Each core runs the
full per-sample block chain for its direction (chunked-SSD form of the
selective scan); the bidirectional combine after layer 1 is a pairwise
AllGather + on-chip average. Direction needs no data reversal on device:
bwd cores get chunk-reversed input (host side), anticausal masks / suffix
cumsums via per-core constant data, and a dynamic conv tap offset.

All big GEMMs run bf16 x bf16 (weights converted host-side, activations
produced in bf16); SSM state and residual stream stay fp32.

Self-contained: hardcodes shapes; imports concourse from known paths.
"""
import os
import sys

for _p in ("/opt/trn_rl_repo", "/root/.axon_site/_ro/trn_rl_repo"):
    if _p not in sys.path:
        sys.path.append(_p)

import numpy as np
import ml_dtypes
import concourse.bass as bass
import concourse.bacc as bacc
import concourse.mybir as mybir
from concourse import tile
from concourse.bass_utils import run_bass_kernel_spmd

F32 = mybir.dt.float32
F32R = mybir.dt.float32r
BF16 = mybir.dt.bfloat16
I32 = mybir.dt.int32
AF = mybir.ActivationFunctionType
OP = mybir.AluOpType

T, D = 1024, 512
H, Q, NCH = 16, 128, 8
D_INNER = 1024
D_INPROJ = 2192
MPAD = 2304
FFN = 2048
EPS = 1e-5
FMAX = 3.0e38
BF = ml_dtypes.bfloat16

_CACHE = {}
PH = int(os.environ.get("KPH", "99"))


def build_nc():
    nc = bacc.Bacc("TRN2", target_bir_lowering=False, debug=False, num_devices=8)

    x_in = nc.dram_tensor("x_fm", [D, T], F32, kind="ExternalInput")
    wip = nc.dram_tensor("wip", [2, 4, 128, MPAD], BF16, kind="ExternalInput")
    wop = nc.dram_tensor("wop", [2, 8, 128, D], BF16, kind="ExternalInput")
    wf1 = nc.dram_tensor("wf1", [2, 4, 128, FFN], BF16, kind="ExternalInput")
    wf2 = nc.dram_tensor("wf2", [2, 16, 128, D], BF16, kind="ExternalInput")
    bf1 = nc.dram_tensor("bf1", [2, 128, 16], F32, kind="ExternalInput")
    bf2 = nc.dram_tensor("bf2", [2, 128, 4], F32, kind="ExternalInput")
    wcv = nc.dram_tensor("wcv", [2, 128, 63], F32, kind="ExternalInput")
    bcv = nc.dram_tensor("bcv", [2, 128, 9], F32, kind="ExternalInput")
    lnw = nc.dram_tensor("lnw", [2, 2, 128, 4], F32, kind="ExternalInput")
    gwc = nc.dram_tensor("gwc", [2, 128, 8], F32, kind="ExternalInput")
    dcol = nc.dram_tensor("dcol", [2, 128, 8], F32, kind="ExternalInput")
    dtbt = nc.dram_tensor("dtbt", [32, 2], F32, kind="ExternalInput")
    neat = nc.dram_tensor("neat", [32, 2], F32, kind="ExternalInput")
    abd = nc.dram_tensor("abd", [32, 3], F32, kind="ExternalInput")
    causal = nc.dram_tensor("causal", [128, 128], F32, kind="ExternalInput")
    identh = nc.dram_tensor("identh", [128, 128], BF16, kind="ExternalInput")
    onesh = nc.dram_tensor("onesh", [1, 128], F32, kind="ExternalInput")
    onehot = nc.dram_tensor("onehot", [128, 1], F32, kind="ExternalInput")
    onescol = nc.dram_tensor("onescol", [128, 1], F32, kind="ExternalInput")
    hotsel = nc.dram_tensor("hotsel", [64, 2048], F32, kind="ExternalInput")
    selcol = nc.dram_tensor("selcol", [128, 2], F32, kind="ExternalInput")
    onesdh = nc.dram_tensor("onesdh", [32, 128], F32, kind="ExternalInput")
    epsh = nc.dram_tensor("epsh", [1, 1], F32, kind="ExternalInput")
    out_t = nc.dram_tensor("out_fm", [D, T], F32, kind="ExternalOutput")

    with tile.TileContext(nc) as tc:
        _emit(nc, tc, locals())
    nc.compile()
    return nc


def _emit(nc, tc, t_):
    x_in, out_t = t_["x_in"], t_["out_t"]
    wip, wop, wf1, wf2 = t_["wip"], t_["wop"], t_["wf1"], t_["wf2"]

    const = tc.alloc_tile_pool(name="const", bufs=1)
    dram = tc.alloc_tile_pool(name="dram", bufs=1, space="DRAM")

    # ---------------- constants ----------------
    def load_const(name, shape, src_ap, dtype=F32):
        t = const.tile(shape, dtype, name=name)
        nc.sync.dma_start(t[:], src_ap)
        return t

    x_res = [const.tile([128, T], F32, name=f"xres{i}") for i in range(4)]
    for i in range(4):
        nc.sync.dma_start(x_res[i][:], x_in.ap()[128 * i:128 * (i + 1), :])

    # preloaded bf16 weights: layer 0 all + layer 1 in_proj at kernel start;
    # layer 1 out_proj/FFN staged at layer-1 start (SBUF headroom)
    pwA = tc.alloc_tile_pool(name="pwA", bufs=1, side="left")
    wipb0 = [pwA.tile([128, MPAD], BF16, name=f"wipb0{k}") for k in range(4)]
    wopb0 = [pwA.tile([128, D], BF16, name=f"wopb0{k}") for k in range(8)]
    wf1b0 = [pwA.tile([128, FFN], BF16, name=f"wf1b0{k}") for k in range(4)]
    wf2b0 = [pwA.tile([128, D], BF16, name=f"wf2b0{k}") for k in range(16)]
    for k in range(4):
        nc.sync.dma_start(wipb0[k][:], wip.ap()[0, k])
    for k in range(8):
        nc.sync.dma_start(wopb0[k][:], wop.ap()[0, k])
    for k in range(4):
        nc.sync.dma_start(wf1b0[k][:], wf1.ap()[0, k])
    for k in range(16):
        nc.sync.dma_start(wf2b0[k][:], wf2.ap()[0, k])

    causal_t = load_const("causal_t", [128, 128], t_["causal"].ap())
    ident_b = const.tile([128, 128], BF16, name="ident_b")
    nc.sync.dma_start(ident_b[:], t_["identh"].ap())
    ones_f = load_const("ones_f", [1, 128], t_["onesh"].ap())
    ones_r = const.tile([1, 128], F32R, name="ones_r")
    nc.vector.tensor_copy(ones_r[:], ones_f[:])
    onehot_f = load_const("onehot_f", [128, 1], t_["onehot"].ap())
    onehot_r = const.tile([128, 1], F32R, name="onehot_r")
    nc.vector.tensor_copy(onehot_r[:], onehot_f[:])
    onescol_f = load_const("onescol_f", [128, 1], t_["onescol"].ap())
    onescol_b = const.tile([128, 1], BF16, name="onescol_b")
    nc.vector.tensor_copy(onescol_b[:], onescol_f[:])
    hotsel_f = load_const("hotsel_f", [64, 2048], t_["hotsel"].ap())
    hotsel_r = const.tile([64, 2048], F32R, name="hotsel_r")
    nc.vector.tensor_copy(hotsel_r[:], hotsel_f[:])
    dtb_t = load_const("dtb_t", [32, 2], t_["dtbt"].ap())
    nea_t = load_const("nea_t", [32, 2], t_["neat"].ap())
    abd_t = load_const("abd_t", [32, 3], t_["abd"].ap())
    lnw_sb = const.tile([128, 16], F32, name="lnw_sb")
    nc.sync.dma_start(lnw_sb.rearrange("p (a b c) -> p a b c", a=2, b=2),
                      t_["lnw"].ap().rearrange("a b p c -> p a b c"))
    gwc_sb = const.tile([128, 16], F32, name="gwc_sb")
    nc.sync.dma_start(gwc_sb.rearrange("p (a c) -> p a c", a=2),
                      t_["gwc"].ap().rearrange("a p c -> p a c"))
    dcol_sb = const.tile([128, 16], F32, name="dcol_sb")
    nc.sync.dma_start(dcol_sb.rearrange("p (a c) -> p a c", a=2),
                      t_["dcol"].ap().rearrange("a p c -> p a c"))
    wcv_sb = const.tile([128, 126], F32, name="wcv_sb")
    nc.sync.dma_start(wcv_sb.rearrange("p (a c) -> p a c", a=2),
                      t_["wcv"].ap().rearrange("a p c -> p a c"))
    bcv_sb = const.tile([128, 18], F32, name="bcv_sb")
    nc.sync.dma_start(bcv_sb.rearrange("p (a c) -> p a c", a=2),
                      t_["bcv"].ap().rearrange("a p c -> p a c"))
    bf1_sb = const.tile([128, 32], F32, name="bf1_sb")
    nc.sync.dma_start(bf1_sb.rearrange("p (a c) -> p a c", a=2),
                      t_["bf1"].ap().rearrange("a p c -> p a c"))
    bf2_sb = const.tile([128, 8], F32, name="bf2_sb")
    nc.sync.dma_start(bf2_sb.rearrange("p (a c) -> p a c", a=2),
                      t_["bf2"].ap().rearrange("a p c -> p a c"))
    sel_t = load_const("sel_t", [128, 2], t_["selcol"].ap())

    eps1 = load_const("eps1", [1, 1], t_["epsh"].ap())
    onesd_c = load_const("onesd_c", [32, 128], t_["onesdh"].ap())
    hT = const.tile([64, 1024], F32R, name="hT")
    hTb = const.tile([64, 1024], BF16, name="hTb")

    cc_in = dram.tile([D, T], BF16, name="cc_in")
    cc_out = dram.tile([2, D, T], BF16, name="cc_out")

    ones_c1 = onescol_b[:, 0:1]   # [K=128, M=1] for partition-sum matmuls

    def _ln(l, which, out_pool):
        """feature-dim LayerNorm of x_res -> 4 bf16 tiles in out_pool."""
        w_col = lnw_sb[:, (l * 2 + which) * 4:(l * 2 + which) * 4 + 4]
        outs = [out_pool.tile([128, T], BF16, tag=f"hln{i}", name=f"hln{i}")
                for i in range(4)]
        px = tc.alloc_tile_pool(name=f"ln{l}{which}x", bufs=2)
        pp = tc.alloc_tile_pool(name=f"ln{l}{which}p", bufs=2, space="PSUM")
        xr, x2 = [], []
        for i in range(4):
            a = px.tile([128, T], BF16, tag="xr", bufs=4, name=f"xr{i}")
            nc.vector.tensor_copy(a[:], x_res[i][:])
            b = px.tile([128, T], BF16, tag="x2", bufs=4, name=f"x2{i}")
            nc.scalar.activation(b[:], x_res[i][:], AF.Square)
            xr.append(a)
            x2.append(b)
        m_row = px.tile([1, T], F32R, bufs=1, name="m_row")
        r_rowr = px.tile([1, T], F32R, bufs=1, name="r_rowr")
        for tb in range(2):
            sl = slice(512 * tb, 512 * (tb + 1))
            ps = pp.tile([1, 512], F32, tag="st", name="st")
            for k in range(4):
                nc.tensor.matmul(ps[:], ones_c1, xr[k][:, sl],
                                 start=(k == 0), stop=(k == 3))
            ps2 = pp.tile([1, 512], F32, tag="st2", name="st2")
            for k in range(4):
                nc.tensor.matmul(ps2[:], ones_c1, x2[k][:, sl],
                                 start=(k == 0), stop=(k == 3))
            nc.scalar.activation(m_row[0:1, sl], ps[:], AF.Copy, scale=1.0 / D)
            msq = px.tile([1, 512], F32, tag="msq", name="msq")
            nc.scalar.activation(msq[:], m_row[0:1, sl], AF.Square)
            var = px.tile([1, 512], F32, tag="var", name="var")
            nc.vector.scalar_tensor_tensor(out=var[:], in0=ps2[:],
                                           scalar=1.0 / D, in1=msq[:],
                                           op0=OP.mult, op1=OP.subtract)
            sq = px.tile([1, 512], F32, tag="sq", name="sq")
            nc.scalar.activation(sq[:], var[:], AF.Sqrt, bias=eps1[:])
            rr = px.tile([1, 512], F32, tag="rr", name="rr")
            nc.vector.reciprocal(rr[:], sq[:])
            nc.vector.tensor_copy(r_rowr[0:1, sl], rr[:])
        for i in range(4):
            for tb in range(2):
                sl = slice(512 * tb, 512 * (tb + 1))
                mb = pp.tile([128, 512], F32, tag="mb", name="mb")
                nc.tensor.matmul(mb[:], ones_r[0:1, :], m_row[0:1, sl],
                                 start=True, stop=True)
                rb = pp.tile([128, 512], F32, tag="rb", name="rb")
                nc.tensor.matmul(rb[:], ones_r[0:1, :], r_rowr[0:1, sl],
                                 start=True, stop=True)
                tmp = px.tile([128, 512], F32, tag="tmp", name="tmp")
                nc.vector.tensor_tensor(out=tmp[:], in0=x_res[i][:, sl],
                                        in1=mb[:], op=OP.subtract)
                nc.vector.scalar_tensor_tensor(
                    out=outs[i][:, sl], in0=tmp[:], scalar=w_col[:, i:i + 1],
                    in1=rb[:], op0=OP.mult, op1=OP.mult)
        pp.release()
        px.release()
        return outs

    # ======================= layers =======================
    for l in range(1 if PH < 10 else 2):
        # layer-1 out_proj/FFN weights staged at layer start (DMA overlaps
        # the mixer phases)
        if l == 1:
            pwA.release()
            pwB = tc.alloc_tile_pool(name="pwB", bufs=1, side="left")
            wipb = [pwB.tile([128, MPAD], BF16, name=f"wipb1{k}")
                    for k in range(4)]
            wopb = [pwB.tile([128, D], BF16, name=f"wopb1{k}")
                    for k in range(8)]
            wf1b = [pwB.tile([128, FFN], BF16, name=f"wf1b1{k}")
                    for k in range(4)]
            wf2b = [pwB.tile([128, D], BF16, name=f"wf2b1{k}")
                    for k in range(16)]
            for k in range(4):
                nc.sync.dma_start(wipb[k][:], wip.ap()[1, k])
            for k in range(8):
                nc.sync.dma_start(wopb[k][:], wop.ap()[1, k])
            for k in range(4):
                nc.sync.dma_start(wf1b[k][:], wf1.ap()[1, k])
            for k in range(16):
                nc.sync.dma_start(wf2b[k][:], wf2.ap()[1, k])
        else:
            wipb, wopb, wf1b, wf2b = wipb0, wopb0, wf1b0, wf2b0

        # two-sided stack pool management (LIFO per side)
        pCz = tc.alloc_tile_pool(name=f"Cz{l}", bufs=1, side="left")
        pF1 = tc.alloc_tile_pool(name=f"F1{l}", bufs=1, side="right")
        pA = tc.alloc_tile_pool(name=f"A{l}", bufs=1, side="right")
        pH = tc.alloc_tile_pool(name=f"H{l}", bufs=1, side="left")

        z_t = [pCz.tile([128, T], BF16, tag=f"zt{i}", name=f"z{i}")
               for i in range(8)]
        xpad = [pA.tile([128, NCH, 134], BF16, tag=f"xpad{f}", name=f"xpad{f}")
                for f in range(9)]
        dtr = pF1.tile([32, T], F32, name="dtr")
        nc.vector.memset(dtr[:], 0.0)

        if PH <= 0:
            pH.release()
            pA.release()
            pCz.release()
            pF1.release()
            break
        h_ln1 = _ln(l, 0, pH)
        if PH <= 1:
            pH.release()
            pA.release()
            pCz.release()
            pF1.release()
            break

        # ---------------- in_proj ----------------
        pp = tc.alloc_tile_pool(name=f"ipp{l}", bufs=3, space="PSUM")
        for mt in range(18):
            mo = 128 * mt
            for tb in range(2):
                sl = slice(512 * tb, 512 * (tb + 1))
                ps = pp.tile([128, 512], F32, tag="mm", name="ps")
                for k in range(4):
                    nc.tensor.matmul(ps[:], wipb[k][:, mo:mo + 128],
                                     h_ln1[k][:, sl],
                                     start=(k == 0), stop=(k == 3))
                if mt < 8:
                    nc.scalar.activation(z_t[mt][:, sl], ps[:], AF.Silu)
                elif mt < 17:
                    f = mt - 8
                    nc.vector.tensor_copy(
                        xpad[f][:, 4 * tb:4 * (tb + 1), 3:131],
                        ps.rearrange("p (c t) -> p c t", c=4))
                else:
                    nc.vector.tensor_copy(dtr[0:16, sl], ps[0:16, :])
        pp.release()
        pH.release()
        if PH <= 2:
            pA.release()
            pCz.release()
            pF1.release()
            break

        # ---------------- conv + silu ----------------
        pB = tc.alloc_tile_pool(name=f"B{l}", bufs=1, side="left")
        xsil = [pB.tile([128, T], BF16, tag=f"xsil{f}", name=f"xsil{f}")
                for f in range(8)]
        B_t = pB.tile([64, T], BF16, name="B_t")
        C_t = pB.tile([64, T], BF16, name="C_t")
        pcv = tc.alloc_tile_pool(name=f"cv{l}", bufs=2, side="right")
        for f in range(9):
            xp = xpad[f]
            nc.vector.memset(xp[:, 0, 0:3], 0.0)
            nc.vector.memset(xp[:, 0, 131:134], 0.0)
            for c in range(1, NCH):
                nc.vector.tensor_copy(xp[:, c, 0:3], xp[:, c - 1, 128:131])
                nc.vector.tensor_copy(xp[:, c, 131:134], xp[:, c - 1, 3:6])
            acc = pcv.tile([128, NCH, 128], BF16, tag="acc", name="acc")
            for k in range(7):
                w_k = wcv_sb[:, l * 63 + f * 7 + k:l * 63 + f * 7 + k + 1]
                src = xp[:, :, k:k + 128]
                if k == 0:
                    nc.vector.tensor_scalar(
                        out=acc[:], in0=src, scalar1=w_k,
                        scalar2=bcv_sb[:, l * 9 + f:l * 9 + f + 1],
                        op0=OP.mult, op1=OP.add)
                else:
                    nc.vector.scalar_tensor_tensor(
                        out=acc[:], in0=src, scalar=w_k, in1=acc[:],
                        op0=OP.mult, op1=OP.add)
            accf = acc.rearrange("p c t -> p (c t)")
            if f < 8:
                nc.scalar.activation(xsil[f][:], accf, AF.Silu)
            else:
                nc.scalar.activation(B_t[:], accf[0:64, :], AF.Silu)
                nc.scalar.activation(C_t[:], accf[64:128, :], AF.Silu)
        pcv.release()
        pA.release()
        if PH <= 3:
            pB.release()
            pCz.release()
            pF1.release()
            break

        # ---------------- dt / decay family, St, B_tok ----------------
        pCy = tc.alloc_tile_pool(name=f"Cy{l}", bufs=1, side="right")
        y_t = [pCy.tile([128, T], BF16, tag=f"yt{i}", name=f"y{i}")
               for i in range(8)]
        pF2 = tc.alloc_tile_pool(name=f"F2{l}", bufs=1, side="right")
        dt_t = pF2.tile([32, T], F32, name="dt_t")
        log_a = pF2.tile([32, T], F32, name="log_a")
        Lhl = pF2.tile([64, T], F32R, name="Lhl")
        nc.vector.memset(Lhl.bitcast(F32), 0.0)
        bias_tok = pF2.tile([128, NCH * 32], F32, name="bias_tok")
        gam_bc = pF2.tile([64, NCH * 16], F32, name="gam_bc")
        w_all = pF2.tile([128, NCH * 16], F32, name="w_all")

        St_all = [pF2.tile([128, 128], BF16, tag=f"st{c}", name=f"St{c}")
                  for c in range(NCH)]
        B_tok = [pF2.tile([128, 64], BF16, tag=f"bt{c}", name=f"Bt{c}")
                 for c in range(NCH)]

        e_sp = pF2.tile([32, T], F32, name="e_sp")
        nc.scalar.activation(e_sp[:], dtr[:], AF.Exp, bias=dtb_t[:, l:l + 1])
        nc.scalar.activation(dt_t[:], e_sp[:], AF.Ln, bias=1.0)
        nc.vector.tensor_scalar(out=log_a[:], in0=dt_t[:],
                                scalar1=nea_t[:, l:l + 1], scalar2=None,
                                op0=OP.mult)

        pd = tc.alloc_tile_pool(name=f"dtf{l}", bufs=3, side="right")
        pg = tc.alloc_tile_pool(name=f"dtp{l}", bufs=2, space="PSUM")
        for c in range(NCH):
            sl = slice(Q * c, Q * (c + 1))
            la = log_a[:, sl]
            P_t = pd.tile([32, 128], F32, tag="P", name="P")
            nc.vector.tensor_tensor_scan(P_t[:], onesd_c[:], la, 0.0,
                                         OP.mult, OP.add)
            Tt = pd.tile([32, 1], F32, tag="Tt", name="Tt")
            nc.vector.tensor_reduce(out=Tt[:], in_=la,
                                    axis=mybir.AxisListType.X, op=OP.add)
            bTt = pd.tile([32, 1], F32, tag="bTt", name="bTt")
            nc.vector.tensor_scalar(out=bTt[:], in0=Tt[:],
                                    scalar1=abd_t[:, 1:2], scalar2=None,
                                    op0=OP.mult)
            da = pd.tile([32, 128], F32, tag="da", name="da")
            nc.vector.tensor_scalar(out=da[:], in0=la, scalar1=abd_t[:, 2:3],
                                    scalar2=None, op0=OP.mult)
            L_c = pd.tile([32, 128], F32, tag="L", name="L")
            nc.vector.scalar_tensor_tensor(out=L_c[:], in0=P_t[:],
                                           scalar=abd_t[:, 0:1], in1=da[:],
                                           op0=OP.mult, op1=OP.add)
            nc.vector.tensor_scalar(out=L_c[:], in0=L_c[:], scalar1=bTt[:],
                                    scalar2=None, op0=OP.add)
            nc.vector.tensor_copy(Lhl[0:16, sl], L_c[0:16, :])
            nc.vector.tensor_tensor(out=Lhl[32:48, sl], in0=L_c[0:16, :],
                                    in1=Lhl[0:16, sl], op=OP.subtract)
            lndt = pd.tile([32, 128], F32, tag="lndt", name="lndt")
            nc.scalar.activation(lndt[:], dt_t[:, sl], AF.Ln)
            bfm = pd.tile([32, 128], F32, tag="bfm", name="bfm")
            nc.vector.tensor_tensor(out=bfm[:], in0=lndt[:], in1=L_c[:],
                                    op=OP.subtract)
            for kb in range(4):
                nc.vector.transpose(
                    out=bias_tok[32 * kb:32 * (kb + 1), 32 * c:32 * c + 32],
                    in_=bfm[0:32, 32 * kb:32 * kb + 32])
            eL = pd.tile([32, 128], F32, tag="eL", name="eL")
            nc.scalar.activation(eL[:], L_c[:], AF.Exp)
            eLt = pd.tile([128, 32], F32, tag="eLt", name="eLt")
            for kb in range(4):
                nc.vector.transpose(out=eLt[32 * kb:32 * (kb + 1), 0:32],
                                    in_=eL[0:32, 32 * kb:32 * kb + 32])
            eLt_r = pd.tile([128, 16], F32R, tag="eLtr", name="eLt_r")
            nc.vector.tensor_copy(eLt_r[:], eLt[:, 0:16])
            gp = pg.tile([1, 16], F32, tag="gp", name="gp")
            nc.tensor.matmul(gp[:], onehot_r[:, :], eLt_r[:],
                             start=True, stop=True)
            gr = pd.tile([1, 16], F32, tag="gr", name="gr")
            nc.vector.tensor_copy(gr[:], gp[:])
            nc.gpsimd.partition_broadcast(gam_bc[:, 16 * c:16 * (c + 1)], gr[:])
            # chunk-summary weight columns: w[j,h] = exp(Lq_h - L_j + lndt_j)
            Lhf = pd.tile([32, 128], F32, tag="Lhf", name="Lhf")
            nc.vector.tensor_copy(Lhf[:], Lhl[0:32, sl])
            Llf = pd.tile([32, 128], F32, tag="Llf", name="Llf")
            nc.vector.tensor_copy(Llf[:], Lhl[32:64, sl])
            Lht = pd.tile([128, 32], F32, tag="Lht", name="Lht")
            Llt = pd.tile([128, 32], F32, tag="Llt", name="Llt")
            for kb in range(4):
                nc.vector.transpose(out=Lht[32 * kb:32 * (kb + 1), 0:32],
                                    in_=Lhf[0:32, 32 * kb:32 * kb + 32])
                nc.vector.transpose(out=Llt[32 * kb:32 * (kb + 1), 0:32],
                                    in_=Llf[0:32, 32 * kb:32 * kb + 32])
            Lht_r = pd.tile([128, 16], F32R, tag="Lhtr", name="Lht_r")
            nc.vector.tensor_copy(Lht_r[:], Lht[:, 0:16])
            Llt_r = pd.tile([128, 16], F32R, tag="Lltr", name="Llt_r")
            nc.vector.tensor_copy(Llt_r[:], Llt[:, 0:16])
            qp = pg.tile([1, 16], F32, tag="qp", name="qp")
            nc.tensor.matmul(qp[:], onehot_r[:, :], Lht_r[:],
                             start=True, stop=False)
            nc.tensor.matmul(qp[:], onehot_r[:, :], Llt_r[:],
                             start=False, stop=True)
            qr = pd.tile([1, 16], F32, tag="qr", name="qr")
            nc.vector.tensor_copy(qr[:], qp[:])
            lqb = pd.tile([128, 16], F32, tag="lqb", name="lqb")
            nc.gpsimd.partition_broadcast(lqb[:], qr[:])
            wpre = pd.tile([128, 16], F32, tag="wpre", name="wpre")
            nc.vector.tensor_tensor(out=wpre[:], in0=lqb[:],
                                    in1=bias_tok[:, 32 * c:32 * c + 16],
                                    op=OP.add)
            nc.scalar.activation(w_all[:, 16 * c:16 * (c + 1)], wpre[:], AF.Exp)
            stp = pg.tile([128, 128], F32, tag="stp", name="stp")
            nc.tensor.matmul(stp[:], B_t[:, sl], C_t[:, sl],
                             start=True, stop=True)
            nc.vector.scalar_tensor_tensor(out=St_all[c][:], in0=stp[:],
                                           scalar=1.0, in1=causal_t[:],
                                           op0=OP.mult, op1=OP.mult)
            btp = pg.tile([128, 64], BF16, tag="btp", name="btp")
            nc.tensor.transpose(btp[:], B_t[:, sl], ident_b[0:64, 0:64])
            nc.vector.tensor_copy(B_tok[c][:], btp[:])
        pg.release()
        pd.release()
        if PH <= 4:
            pF2.release()
            pCy.release()
            pB.release()
            pCz.release()
            pF1.release()
            break

        # ---------------- scan ----------------
        nc.vector.memset(hT.bitcast(F32), 0.0)
        psc = tc.alloc_tile_pool(name=f"sc{l}", bufs=3, side="left")
        pxt = tc.alloc_tile_pool(name=f"xt{l}", bufs=2, side="left")
        pbc = tc.alloc_tile_pool(name=f"bcp{l}", bufs=2, space="PSUM")
        pyp = tc.alloc_tile_pool(name=f"ypp{l}", bufs=2, space="PSUM")
        psp = tc.alloc_tile_pool(name=f"spp{l}", bufs=1, space="PSUM")
        pxp = tc.alloc_tile_pool(name=f"xtp{l}", bufs=2, space="PSUM")
        for c in range(NCH):
            sl = slice(Q * c, Q * (c + 1))
            xtk = pxt.tile([128, 1024], BF16, tag="xtk", name="xtk")
            for f in range(8):
                tp = pxp.tile([128, 128], BF16, tag="tp", name="tp")
                nc.tensor.transpose(tp[:], xsil[f][:, sl], ident_b[:, :])
                nc.vector.tensor_copy(xtk[:, 128 * f:128 * (f + 1)], tp[:])
            nc.vector.tensor_copy(hTb[:], hT[:])
            sS = psp.tile([64, 1024], F32, tag="sS", name="sS")
            for h in range(H):
                hs = slice(64 * h, 64 * (h + 1))
                bc = pbc.tile([128, 128], F32, tag="bc", name="bc")
                nc.tensor.matmul(bc[:], hotsel_r[:, 128 * h:128 * (h + 1)],
                                 Lhl[:, sl], start=True, stop=True)
                mexp = psc.tile([128, 128], F32, tag="mexp", name="mexp")
                nc.scalar.activation(
                    mexp[:], bc[:], AF.Exp,
                    bias=bias_tok[:, 32 * c + h:32 * c + h + 1])
                stm = psc.tile([128, 128], BF16, tag="stm", name="stm")
                nc.vector.scalar_tensor_tensor(
                    out=stm[:], in0=mexp[:], scalar=FMAX, in1=St_all[c][:],
                    op0=OP.min, op1=OP.mult)
                mst_e = psc.tile([64, 128], F32, tag="mste", name="mste")
                nc.scalar.activation(mst_e[:], bc[0:64, :], AF.Exp)
                mst = psc.tile([64, 128], BF16, tag="mst", name="mst")
                nc.vector.tensor_tensor(out=mst[:], in0=C_t[:, sl],
                                        in1=mst_e[:], op=OP.mult)
                yp = pyp.tile([64, 128], F32, tag="yp", name="yp")
                nc.tensor.matmul(yp[:], hTb[:, hs], mst[:],
                                 start=True, stop=False)
                nc.tensor.matmul(yp[:], xtk[:, hs], stm[:],
                                 start=False, stop=True)
                ft, ro = h // 2, 64 * (h % 2)
                nc.vector.scalar_tensor_tensor(
                    out=y_t[ft][ro:ro + 64, sl],
                    in0=xsil[ft][ro:ro + 64, sl],
                    scalar=dcol_sb[ro:ro + 64, l * 8 + ft:l * 8 + ft + 1],
                    in1=yp[:], op0=OP.mult, op1=OP.add)
                wb = psc.tile([128, 64], BF16, tag="wb", name="wb")
                nc.vector.tensor_scalar(
                    out=wb[:], in0=B_tok[c][:],
                    scalar1=w_all[:, 16 * c + h:16 * c + h + 1], scalar2=None,
                    op0=OP.mult)
                nc.tensor.matmul(sS[:, hs], wb[:], xtk[:, hs],
                                 start=True, stop=True)
            for h in range(H):
                hs = slice(64 * h, 64 * (h + 1))
                nc.vector.scalar_tensor_tensor(
                    out=hT[:, hs], in0=hT[:, hs],
                    scalar=gam_bc[:, 16 * c + h:16 * c + h + 1],
                    in1=sS[:, hs], op0=OP.mult, op1=OP.add)
        pxp.release()
        psp.release()
        pyp.release()
        pbc.release()
        pxt.release()
        psc.release()
        pB.release()
        pF2.release()
        if PH <= 5:
            pCy.release()
            pCz.release()
            pF1.release()
            break

        # ---------------- gating + rmsnorm (in place on y_t) ----------------
        pgt = tc.alloc_tile_pool(name=f"gt{l}", bufs=2, side="left")
        pgp = tc.alloc_tile_pool(name=f"gp{l}", bufs=2, space="PSUM")
        pgb = tc.alloc_tile_pool(name=f"gb{l}", bufs=2, space="PSUM")
        for f in range(8):
            nc.vector.tensor_tensor(out=y_t[f][:], in0=y_t[f][:],
                                    in1=z_t[f][:], op=OP.mult)
        r_rowr = pgt.tile([1, T], F32R, name="grrowr")
        for tb in range(2):
            sl = slice(512 * tb, 512 * (tb + 1))
            ps = pgp.tile([1, 512], F32, tag="gst", name="gst")
            for k in range(8):
                g2 = pgt.tile([128, 512], BF16, tag="g2", bufs=3, name="g2")
                nc.scalar.activation(g2[:], y_t[k][:, sl], AF.Square)
                nc.tensor.matmul(ps[:], ones_c1, g2[:],
                                 start=(k == 0), stop=(k == 7))
            sq = pgt.tile([1, 512], F32, tag="gsq", name="gsq")
            nc.scalar.activation(sq[:], ps[:], AF.Sqrt, bias=eps1[:],
                                 scale=1.0 / D_INNER)
            rr = pgt.tile([1, 512], F32, tag="grr", name="grr")
            nc.vector.reciprocal(rr[:], sq[:])
            nc.vector.tensor_copy(r_rowr[0:1, sl], rr[:])
        for f in range(8):
            for tb in range(2):
                sl = slice(512 * tb, 512 * (tb + 1))
                rb = pgb.tile([128, 512], F32, tag="grb", name="grb")
                nc.tensor.matmul(rb[:], ones_r[0:1, :], r_rowr[0:1, sl],
                                 start=True, stop=True)
                nc.vector.scalar_tensor_tensor(
                    out=y_t[f][:, sl], in0=y_t[f][:, sl],
                    scalar=gwc_sb[:, l * 8 + f:l * 8 + f + 1], in1=rb[:],
                    op0=OP.mult, op1=OP.mult)
        pgb.release()
        pgp.release()
        pgt.release()
        pCz.release()
        if PH <= 6:
            pCy.release()
            pF1.release()
            break

        # ---------------- out_proj (+ residual) ----------------
        pp = tc.alloc_tile_pool(name=f"opp{l}", bufs=3, space="PSUM")
        for mt in range(4):
            for tb in range(2):
                sl = slice(512 * tb, 512 * (tb + 1))
                ps = pp.tile([128, 512], F32, tag="mm", name="ps")
                for k in range(8):
                    nc.tensor.matmul(ps[:], wopb[k][:, 128 * mt:128 * (mt + 1)],
                                     y_t[k][:, sl], start=(k == 0), stop=(k == 7))
                nc.vector.tensor_tensor(out=x_res[mt][:, sl],
                                        in0=x_res[mt][:, sl], in1=ps[:],
                                        op=OP.add)
        pp.release()
        pCy.release()

        # ---------------- FFN ----------------
        pG = tc.alloc_tile_pool(name=f"G{l}", bufs=1, side="left")
        G_t = [pG.tile([128, T], BF16, tag=f"G{i}", name=f"G{i}")
               for i in range(16)]
        pH2 = tc.alloc_tile_pool(name=f"H2{l}", bufs=1, side="left")
        h_ln2 = _ln(l, 1, pH2)
        pp = tc.alloc_tile_pool(name=f"f1p{l}", bufs=3, space="PSUM")
        for mt in range(16):
            for tb in range(2):
                sl = slice(512 * tb, 512 * (tb + 1))
                ps = pp.tile([128, 512], F32, tag="mm", name="ps")
                for k in range(4):
                    nc.tensor.matmul(ps[:], wf1b[k][:, 128 * mt:128 * (mt + 1)],
                                     h_ln2[k][:, sl], start=(k == 0), stop=(k == 3))
                nc.scalar.activation(G_t[mt][:, sl], ps[:], AF.Gelu_apprx_tanh,
                                     bias=bf1_sb[:, l * 16 + mt:l * 16 + mt + 1])
        pp.release()
        pH2.release()

        pp = tc.alloc_tile_pool(name=f"f2p{l}", bufs=3, space="PSUM")
        for mt in range(4):
            for tb in range(2):
                sl = slice(512 * tb, 512 * (tb + 1))
                ps = pp.tile([128, 512], F32, tag="mm", name="ps")
                for k in range(16):
                    nc.tensor.matmul(ps[:], wf2b[k][:, 128 * mt:128 * (mt + 1)],
                                     G_t[k][:, sl], start=(k == 0), stop=(k == 15))
                nc.vector.scalar_tensor_tensor(
                    out=x_res[mt][:, sl], in0=ps[:],
                    scalar=bf2_sb[:, l * 4 + mt:l * 4 + mt + 1],
                    in1=x_res[mt][:, sl], op0=OP.add, op1=OP.add)
        pp.release()
        pG.release()
        pF1.release()
        if l == 1:
            pwB.release()

        # ---------------- pairwise combine after layer 0 ----------------
        if l == 0 and PH >= 9:
            pcc = tc.alloc_tile_pool(name="ccsb", bufs=1)
            stg = pcc.tile([128, 4, T], BF16, name="ccstg")
            for i in range(4):
                nc.vector.tensor_copy(stg[:, i], x_res[i][:])
                nc.sync.dma_start(cc_in[128 * i:128 * (i + 1), :], stg[:, i])
            nc.gpsimd.collective_compute(
                "AllGather", OP.bypass, ins=[cc_in.opt()], outs=[cc_out.opt()],
                replica_groups=[[0, 1], [2, 3], [4, 5], [6, 7]])
            cc_sb = pcc.tile([128, 2, 4, T], BF16, name="ccsb")
            nc.sync.dma_start(cc_sb[:],
                              cc_out.rearrange("a (f p) t -> p a f t", p=128))
            for f in range(4):
                # a[c] = slot0[c] + slot1[7-c]; x1[c] = w0*a[c] + w1*a[7-c]
                a_t = pcc.tile([128, T], BF16, tag=f"cca{f}", name=f"cca{f}")
                for c in range(NCH):
                    nc.vector.tensor_tensor(
                        out=a_t[:, 128 * c:128 * (c + 1)],
                        in0=cc_sb[:, 0, f, 128 * c:128 * (c + 1)],
                        in1=cc_sb[:, 1, f, 128 * (7 - c):128 * (8 - c)],
                        op=OP.add)
                for c in range(NCH):
                    tmp = pcc.tile([128, 128], F32, tag="cct", bufs=3,
                                   name="cct")
                    nc.vector.tensor_scalar(
                        out=tmp[:], in0=a_t[:, 128 * c:128 * (c + 1)],
                        scalar1=sel_t[:, 0:1], scalar2=None, op0=OP.mult)
                    nc.vector.scalar_tensor_tensor(
                        out=x_res[f][:, 128 * c:128 * (c + 1)],
                        in0=a_t[:, 128 * (7 - c):128 * (8 - c)],
                        scalar=sel_t[:, 1:2], in1=tmp[:],
                        op0=OP.mult, op1=OP.add)
            pcc.release()

    for i in range(4):
        nc.sync.dma_start(out_t.ap()[128 * i:128 * (i + 1), :], x_res[i][:])

    if PH < 10:
        pwA.release()
    dram.release()
    const.release()


# ----------------------------------------------------------------------------
# host side
# ----------------------------------------------------------------------------

def _pos_enc():
    pos = np.arange(T, dtype=np.float32)[:, None]
    div = np.exp(-np.log(10000.0) * np.arange(0, D, 2, dtype=np.float32) / D)
    ang = pos * div
    return np.stack([np.sin(ang), np.cos(ang)], axis=-1).reshape(T, D)


def _shuffle_chunks(x_td):
    return np.ascontiguousarray(
        x_td.reshape(NCH, Q, *x_td.shape[1:])[::-1].reshape(x_td.shape))


def _core_inputs(inputs, b, d):
    f32 = np.float32
    x = np.asarray(inputs["x"], f32)[b] + _pos_enc()
    if d == 1:
        x = _shuffle_chunks(x)
    im = {"x_fm": np.ascontiguousarray(x.T)}
    ls = [d, 2 + d]
    wip_ = np.zeros((2, D, MPAD), f32)
    for i, j in enumerate(ls):
        wip_[i, :, :D_INPROJ] = np.asarray(inputs["in_proj_w"], f32)[j]
    im["wip"] = np.ascontiguousarray(wip_.reshape(2, 4, 128, MPAD)).astype(BF)
    im["wop"] = np.ascontiguousarray(
        np.asarray(inputs["out_proj_w"], f32)[ls].reshape(2, 8, 128, D)).astype(BF)
    im["wf1"] = np.ascontiguousarray(
        np.asarray(inputs["ffn_w1"], f32)[ls].reshape(2, 4, 128, FFN)).astype(BF)
    im["wf2"] = np.ascontiguousarray(
        np.asarray(inputs["ffn_w2"], f32)[ls].reshape(2, 16, 128, D)).astype(BF)
    im["bf1"] = np.ascontiguousarray(
        np.asarray(inputs["ffn_b1"], f32)[ls].reshape(2, 16, 128).transpose(0, 2, 1))
    im["bf2"] = np.ascontiguousarray(
        np.asarray(inputs["ffn_b2"], f32)[ls].reshape(2, 4, 128).transpose(0, 2, 1))
    cw = np.asarray(inputs["conv_w"], f32)[ls]          # [2, 4, 1152]
    cw7 = np.zeros((2, 7, 1152), f32)
    if d == 0:
        cw7[:, 0:4] = cw
    else:
        cw7[:, 3:7] = cw[:, ::-1, :]
    im["wcv"] = np.ascontiguousarray(
        cw7.reshape(2, 7, 9, 128).transpose(0, 3, 2, 1).reshape(2, 128, 63))
    im["bcv"] = np.ascontiguousarray(
        np.asarray(inputs["conv_b"], f32)[ls].reshape(2, 9, 128).transpose(0, 2, 1))
    lnwa = np.stack([np.asarray(inputs["ln1_w"], f32)[ls],
                     np.asarray(inputs["ln2_w"], f32)[ls]], axis=1)
    im["lnw"] = np.ascontiguousarray(
        lnwa.reshape(2, 2, 4, 128).transpose(0, 1, 3, 2))
    im["gwc"] = np.ascontiguousarray(
        np.asarray(inputs["gnorm_w"], f32)[ls].reshape(2, 8, 128).transpose(0, 2, 1))
    Dp = np.asarray(inputs["Dparam"], f32)[ls]
    im["dcol"] = np.ascontiguousarray(
        np.repeat(Dp, 64, axis=1).reshape(2, 8, 128).transpose(0, 2, 1))
    dtb = np.zeros((32, 2), f32)
    dtb[:16] = np.asarray(inputs["dt_bias"], f32)[ls].T
    im["dtbt"] = dtb
    nea = np.zeros((32, 2), f32)
    nea[:16] = -np.exp(np.asarray(inputs["A_log"], f32)[ls]).T
    im["neat"] = nea
    abdv = np.zeros((32, 3), f32)
    if d == 0:
        abdv[:, 0] = 1.0
    else:
        abdv[:, 0] = -1.0
        abdv[:, 1] = 1.0
        abdv[:, 2] = 1.0
    im["abd"] = abdv
    jj, ii = np.meshgrid(np.arange(Q), np.arange(Q), indexing="ij")
    im["causal"] = ((jj <= ii) if d == 0 else (jj >= ii)).astype(f32)
    im["identh"] = np.eye(128, dtype=f32).astype(BF)
    im["onesh"] = np.ones((1, 128), f32)
    oh = np.zeros((128, 1), f32)
    oh[127 if d == 0 else 0] = 1.0
    im["onehot"] = oh
    im["onescol"] = np.ones((128, 1), f32)
    im["onesdh"] = np.ones((32, 128), f32)
    im["epsh"] = np.full((1, 1), EPS, f32)
    hs_ = np.zeros((64, 16, 128), f32)
    for hh in range(16):
        hs_[hh, hh, :] = 1.0
        hs_[32 + hh, hh, :] = 1.0
    im["hotsel"] = np.ascontiguousarray(hs_.reshape(64, 2048))
    sel = np.zeros((128, 2), f32)
    sel[:, 0 if d == 0 else 1] = 0.5
    im["selcol"] = sel
    return im


def _get_nc():
    if "nc" not in _CACHE:
        _CACHE["nc"] = build_nc()
    return _CACHE["nc"]


def kernel(**inputs):
    nc = _get_nc()
    in_maps = [_core_inputs(inputs, c // 2, c % 2) for c in range(8)]
    res = run_bass_kernel_spmd(nc, in_maps, list(range(8)))
    out = np.zeros((4, T, D), np.float32)
    for b in range(4):
        fwd = res.results[2 * b]["out_fm"].T
        bwd = _shuffle_chunks(np.ascontiguousarray(res.results[2 * b + 1]["out_fm"].T))
        out[b] = 0.5 * (fwd + bwd)
    lengths = np.asarray(inputs["lengths"])
    mask = (np.arange(T)[None, :] < lengths[:, None]).astype(np.float32)
    return (out * mask[:, :, None]).astype(np.float32)


if __name__ == "__main__":
    print("building...")
    _get_nc()
    print("built ok")
